# revision 6
# baseline (speedup 1.0000x reference)
"""MoE decoder Trainium2 kernel (nn_MoEDecoder_67654324846797).

Strategy
--------
Data-parallel: the token dim (N=65536) is sharded across 8 NeuronCores
(8192 tokens each); all weights are replicated. No collectives.

Per-core kernel (feature-major, weight-stationary, f32r matmuls):
  - x tiles are loaded token-major and transposed on the PE (16x [128,128]
    per 512-token tile) into feature-major xT [512, 512tok].
  - All matmuls run with weights as the stationary operand in float32r
    (fp32 storage rounded to 11 mantissa bits, fp32 PSUM accumulation;
    ~1.6e-4 end-to-end rel err, same speed as bf16 on this stack).
  - Gating: 3-layer MLP -> logits l.T [8, 512] in PSUM; softmax is done
    without max subtraction (logits for this model are in [-0.5, 0.35]):
    exp on ACT, sum over experts via a ones-matmul, reciprocal on DVE.
  - exp_e rows are partition-broadcast (GPSIMD) to [128,512] and applied
    to h2_e on DVE; layer-3 matmuls of all 8 experts then accumulate into
    one PSUM bank pair, plus a K=8 bias matmul eb3.T @ expT.
  - The summed output is scaled by 1/Z (broadcast) and transposed back to
    token-major on the PE, then DMA'd out.
"""

import numpy as np

import concourse.bass as bass
import concourse.tile as tile
from concourse import bacc, mybir
from concourse.masks import make_identity

F32 = mybir.dt.float32
F32R = mybir.dt.float32r

N_TOKENS = 65536
N_CORES = 8
TOK_PER_CORE = N_TOKENS // N_CORES  # 8192
TILE = 512  # tokens per tile
N_TILES = TOK_PER_CORE // TILE  # 16
IN_CH = 512
HID = 256
OUT_CH = 256
E = 8

RELU = mybir.ActivationFunctionType.Relu
EXP = mybir.ActivationFunctionType.Exp


def build_kernel(time_reps: int = 1) -> bass.Bass:
    """Build the per-core SPMD program. time_reps>1 wraps the main loop in a
    hardware repeat loop (same work each iteration) for timing."""
    nc = bacc.Bacc("TRN2", target_bir_lowering=False, debug=False,
                   num_devices=N_CORES)

    x = nc.dram_tensor("x", [TOK_PER_CORE, IN_CH], F32R, kind="ExternalInput").ap()
    eW1 = nc.dram_tensor("eW1", [E, IN_CH, HID], F32R, kind="ExternalInput").ap()
    eb1 = nc.dram_tensor("eb1", [E, HID], F32, kind="ExternalInput").ap()
    eW2 = nc.dram_tensor("eW2", [E, HID, HID], F32R, kind="ExternalInput").ap()
    eb2 = nc.dram_tensor("eb2", [E, HID], F32, kind="ExternalInput").ap()
    eW3 = nc.dram_tensor("eW3", [E, HID, OUT_CH], F32R, kind="ExternalInput").ap()
    eb3 = nc.dram_tensor("eb3", [E, OUT_CH], F32R, kind="ExternalInput").ap()
    gW1 = nc.dram_tensor("gW1", [IN_CH, HID], F32R, kind="ExternalInput").ap()
    gb1 = nc.dram_tensor("gb1", [HID], F32, kind="ExternalInput").ap()
    gW2 = nc.dram_tensor("gW2", [HID, HID], F32R, kind="ExternalInput").ap()
    gb2 = nc.dram_tensor("gb2", [HID], F32, kind="ExternalInput").ap()
    gW3 = nc.dram_tensor("gW3", [HID, E], F32R, kind="ExternalInput").ap()
    gb3 = nc.dram_tensor("gb3", [E], F32, kind="ExternalInput").ap()
    out = nc.dram_tensor("out", [TOK_PER_CORE, OUT_CH], F32, kind="ExternalOutput").ap()

    with tile.TileContext(nc) as tc:
        _body(nc, tc, x, eW1, eb1, eW2, eb2, eW3, eb3,
              gW1, gb1, gW2, gb2, gW3, gb3, out, time_reps)
    nc.compile()
    return nc


def _body(nc, tc, x, eW1, eb1, eW2, eb2, eW3, eb3,
          gW1, gb1, gW2, gb2, gW3, gb3, out, time_reps):
    from contextlib import ExitStack

    ctx = ExitStack()
    with ctx:
        wpool = ctx.enter_context(tc.tile_pool(name="wpool", bufs=1))
        io_pool = ctx.enter_context(tc.tile_pool(name="io", bufs=2))
        act_pool = ctx.enter_context(tc.tile_pool(name="act", bufs=2))
        small_pool = ctx.enter_context(tc.tile_pool(name="small", bufs=2))
        ps_mlp = ctx.enter_context(tc.tile_pool(name="ps_mlp", bufs=3, space="PSUM"))
        ps_out = ctx.enter_context(tc.tile_pool(name="ps_out", bufs=1, space="PSUM"))
        ps_tr = ctx.enter_context(tc.tile_pool(name="ps_tr", bufs=2, space="PSUM"))
        ps_small = ctx.enter_context(tc.tile_pool(name="ps_small", bufs=1, space="PSUM"))
        dram_pool = ctx.enter_context(tc.tile_pool(name="dram", bufs=2, space="DRAM"))

        # ---- weight preload (feature-major, stationary layouts) ----
        # lhsT tiles are [K=128 part, M free]; K = input-feature chunks.
        w1e = wpool.tile([128, E, 4, HID], F32R, name="w1e")
        nc.sync.dma_start(w1e, eW1.rearrange("e (kt kp) m -> kp e kt m", kp=128))
        w2e = wpool.tile([128, E, 2, HID], F32R, name="w2e")
        nc.sync.dma_start(w2e, eW2.rearrange("e (kt kp) m -> kp e kt m", kp=128))
        w3e = wpool.tile([128, E, 2, OUT_CH], F32R, name="w3e")
        nc.sync.dma_start(w3e, eW3.rearrange("e (kt kp) m -> kp e kt m", kp=128))
        g1w = wpool.tile([128, 4, HID], F32R, name="g1w")
        nc.sync.dma_start(g1w, gW1.rearrange("(kt kp) m -> kp kt m", kp=128))
        g2w = wpool.tile([128, 2, HID], F32R, name="g2w")
        nc.sync.dma_start(g2w, gW2.rearrange("(kt kp) m -> kp kt m", kp=128))
        g3w = wpool.tile([128, 2, E], F32R, name="g3w")
        nc.sync.dma_start(g3w, gW3.rearrange("(kt kp) m -> kp kt m", kp=128))

        # biases, feature-major: [128 part(out-feature chunk), mt]
        b1e = wpool.tile([128, E, 2], F32, name="b1e")
        nc.sync.dma_start(b1e, eb1.rearrange("e (mt mp) -> mp e mt", mp=128))
        b2e = wpool.tile([128, E, 2], F32, name="b2e")
        nc.sync.dma_start(b2e, eb2.rearrange("e (mt mp) -> mp e mt", mp=128))
        b3e = wpool.tile([E, OUT_CH], F32R, name="b3e")  # lhsT for bias matmul
        nc.sync.dma_start(b3e, eb3)
        g1b = wpool.tile([128, 2], F32, name="g1b")
        nc.sync.dma_start(g1b, gb1.rearrange("(mt mp) -> mp mt", mp=128))
        g2b = wpool.tile([128, 2], F32, name="g2b")
        nc.sync.dma_start(g2b, gb2.rearrange("(mt mp) -> mp mt", mp=128))
        g3b = wpool.tile([E, 1], F32, name="g3b")
        nc.sync.dma_start(g3b, gb3.rearrange("(e one) -> e one", one=1))

        identf = wpool.tile([128, 128], F32, name="identf")
        make_identity(nc, identf)
        identr = wpool.tile([128, 128], F32R, name="identr")
        nc.vector.tensor_copy(identr, identf)
        ones8 = wpool.tile([E, 1], F32, name="ones8")
        nc.vector.memset(ones8, 1.0)
        ones8r = wpool.tile([E, 1], F32R, name="ones8r")
        nc.vector.tensor_copy(ones8r, ones8)

        x_r = x.rearrange("(t s p) f -> t p s f", p=128, s=4)  # [16,128,4,512]
        out_r = out.rearrange("(t s p) o -> t p s o", p=128, s=4)

        def tile_body(t):
            # ---- load + transpose x ----
            x_nat = io_pool.tile([128, 4, IN_CH], F32R, name="x_nat")
            nc.sync.dma_start(x_nat, x_r[t])
            xT = act_pool.tile([128, 4, TILE], F32R, name="xT")
            for kt in range(4):
                p_tr = ps_tr.tile([128, TILE], F32R, name="p_tr", tag="ptr")
                for s in range(4):
                    nc.tensor.transpose(
                        p_tr[:, s * 128:(s + 1) * 128],
                        x_nat[:, s, kt * 128:(kt + 1) * 128], identr)
                nc.vector.tensor_copy(xT[:, kt, :], p_tr)

            # ---- gating MLP ----
            g1T = act_pool.tile([128, 2, TILE], F32R, name="g1T", bufs=1)
            for mt in range(2):
                p_g = ps_mlp.tile([128, TILE], F32, name="p_g", tag="pmlp")
                for kt in range(4):
                    nc.tensor.matmul(p_g, g1w[:, kt, mt * 128:(mt + 1) * 128],
                                     xT[:, kt, :], start=(kt == 0), stop=(kt == 3))
                nc.scalar.activation(g1T[:, mt, :], p_g, RELU, bias=g1b[:, mt:mt + 1])
            g2T = act_pool.tile([128, 2, TILE], F32R, name="g2T", bufs=1)
            for mt in range(2):
                p_g2 = ps_mlp.tile([128, TILE], F32, name="p_g2", tag="pmlp")
                for kt in range(2):
                    nc.tensor.matmul(p_g2, g2w[:, kt, mt * 128:(mt + 1) * 128],
                                     g1T[:, kt, :], start=(kt == 0), stop=(kt == 1))
                nc.scalar.activation(g2T[:, mt, :], p_g2, RELU, bias=g2b[:, mt:mt + 1])
            p_l = ps_small.tile([E, TILE], F32, name="p_l", tag="psmall")
            for kt in range(2):
                nc.tensor.matmul(p_l, g3w[:, kt, :], g2T[:, kt, :],
                                 start=(kt == 0), stop=(kt == 1))
            expT = small_pool.tile([E, TILE], F32R, name="expT")
            nc.scalar.activation(expT, p_l, EXP, bias=g3b)

            # Z = sum_e exp_e  (ones matmul), r = 1/Z
            p_z = ps_small.tile([1, TILE], F32, name="p_z", tag="psmall")
            nc.tensor.matmul(p_z, ones8r, expT, start=True, stop=True)
            r_sb = small_pool.tile([1, TILE], F32, name="r_sb")
            nc.vector.reciprocal(r_sb, p_z)
            exp_dram = dram_pool.tile([E, TILE], F32R, name="exp_dram")
            nc.sync.dma_start(exp_dram, expT)
            r_dram = dram_pool.tile([1, TILE], F32, name="r_dram")
            nc.sync.dma_start(r_dram, r_sb)
            r_bc = act_pool.tile([128, TILE], F32, name="r_bc")
            nc.gpsimd.dma_start(r_bc, r_dram[0, :].partition_broadcast(128))
            w_bc = act_pool.tile([128, E, TILE], F32R, name="w_bc", bufs=1)
            nc.gpsimd.dma_start(w_bc, exp_dram.partition_broadcast(128))

            # ---- experts ----
            p_o = [ps_out.tile([128, TILE], F32, name=f"p_o{mt}", tag=f"po{mt}") for mt in range(2)]
            for e in range(E):
                h1T = act_pool.tile([128, 2, TILE], F32R, name="h1T")
                for mt in range(2):
                    p_h = ps_mlp.tile([128, TILE], F32, name="p_h", tag="pmlp")
                    for kt in range(4):
                        nc.tensor.matmul(p_h, w1e[:, e, kt, mt * 128:(mt + 1) * 128],
                                         xT[:, kt, :], start=(kt == 0), stop=(kt == 3))
                    nc.scalar.activation(h1T[:, mt, :], p_h, RELU,
                                         bias=b1e[:, e, mt:mt + 1])
                h2s = act_pool.tile([128, 2, TILE], F32R, name="h2s")
                for mt in range(2):
                    p_h2 = ps_mlp.tile([128, TILE], F32, name="p_h2", tag="pmlp")
                    for kt in range(2):
                        nc.tensor.matmul(p_h2, w2e[:, e, kt, mt * 128:(mt + 1) * 128],
                                         h1T[:, kt, :], start=(kt == 0), stop=(kt == 1))
                    h2T = act_pool.tile([128, TILE], F32R, name="h2T")
                    nc.scalar.activation(h2T, p_h2, RELU, bias=b2e[:, e, mt:mt + 1])
                    nc.vector.tensor_mul(h2s[:, mt, :], h2T, w_bc[:, e, :])
                for mt in range(2):
                    for kt in range(2):
                        nc.tensor.matmul(p_o[mt], w3e[:, e, kt, mt * 128:(mt + 1) * 128],
                                         h2s[:, kt, :],
                                         start=(e == 0 and kt == 0), stop=False,
                                         skip_group_check=True)

            # gated bias: p_o[mt] += eb3.T[mt-slice] @ expT
            for mt in range(2):
                nc.tensor.matmul(p_o[mt], b3e[:, mt * 128:(mt + 1) * 128], expT,
                                 start=False, stop=True, skip_group_check=True)

            # normalize and transpose out
            outT = act_pool.tile([128, 2, TILE], F32, name="outT")
            for mt in range(2):
                nc.vector.tensor_mul(outT[:, mt, :], p_o[mt], r_bc)
            out_tok = io_pool.tile([128, 4, OUT_CH], F32, name="out_tok")
            for s in range(4):
                p_ot = ps_tr.tile([128, OUT_CH], F32, name="p_ot", tag="ptr")
                for mt in range(2):
                    nc.tensor.transpose(
                        p_ot[:, mt * 128:(mt + 1) * 128],
                        outT[:, mt, s * 128:(s + 1) * 128], identf)
                nc.vector.tensor_copy(out_tok[:, s, :], p_ot)
            nc.sync.dma_start(out_r[t], out_tok)

        if time_reps > 1:
            with tc.For_i(0, time_reps, 1):
                for t in range(N_TILES):
                    tile_body(t)
        else:
            for t in range(N_TILES):
                tile_body(t)


# ---------------------------------------------------------------------------
# PJRT runner (self-contained; mirrors concourse.bass2jax.run_bass_via_pjrt
# but keeps the jitted callable + device inputs for repeat timing)
# ---------------------------------------------------------------------------
class BassRunner:
    def __init__(self, nc: bass.Bass, n_cores: int = 8):
        import jax
        from jax.sharding import Mesh, PartitionSpec
        from jax.experimental.shard_map import shard_map
        from concourse.bass2jax import (
            _bass_exec_p, install_neuronx_cc_hook, partition_id_tensor,
        )

        install_neuronx_cc_hook()
        self.jax = jax
        self.nc = nc
        self.n_cores = n_cores
        partition_name = (
            nc.partition_id_tensor.name if nc.partition_id_tensor else None
        )

        in_names, out_names, out_avals, zero_shapes = [], [], [], []
        for alloc in nc.m.functions[0].allocations:
            if not isinstance(alloc, mybir.MemoryLocationSet):
                continue
            name = alloc.memorylocations[0].name
            if alloc.kind == "ExternalInput":
                if name != partition_name:
                    in_names.append(name)
            elif alloc.kind == "ExternalOutput":
                shape = tuple(alloc.tensor_shape)
                np_dt = mybir.dt.np(alloc.dtype)
                out_names.append(name)
                out_avals.append(jax.core.ShapedArray(shape, np_dt))
                zero_shapes.append((shape, np_dt))

        self.in_names, self.out_names = in_names, out_names
        self.out_avals, self.zero_shapes = out_avals, zero_shapes
        n_params, n_outs = len(in_names), len(out_names)
        bind_in_names = in_names + out_names
        if partition_name is not None:
            bind_in_names.append(partition_name)

        def _b(*args):
            operands = list(args)
            if partition_name is not None:
                operands.append(partition_id_tensor())
            return tuple(_bass_exec_p.bind(
                *operands,
                out_avals=tuple(out_avals),
                in_names=tuple(bind_in_names),
                out_names=tuple(out_names),
                lowering_input_output_aliases=(),
                sim_require_finite=True,
                sim_require_nnan=True,
                nc=nc,
            ))

        devices = jax.devices()[:n_cores]
        assert len(devices) == n_cores
        self.mesh = Mesh(np.asarray(devices), ("core",))
        self.pspec = PartitionSpec("core")
        in_specs = (self.pspec,) * (n_params + n_outs)
        out_specs = (self.pspec,) * n_outs
        self.sharded = jax.jit(
            shard_map(_b, mesh=self.mesh, in_specs=in_specs,
                      out_specs=out_specs, check_rep=False),
            keep_unused=True,
        )
        self._dev_in = None

    def put_inputs(self, in_maps):
        import jax
        concat = [
            np.concatenate([in_maps[c][n] for c in range(self.n_cores)], axis=0)
            for n in self.in_names
        ]
        zeros = [
            np.zeros((self.n_cores * s[0], *s[1:]), d) for s, d in self.zero_shapes
        ]
        sh = jax.sharding.NamedSharding(self.mesh, self.pspec)
        self._dev_in = [jax.device_put(a, sh) for a in concat + zeros]
        jax.block_until_ready(self._dev_in)

    def run(self):
        out = self.sharded(*self._dev_in)
        self.jax.block_until_ready(out)
        return out

    def results(self, out):
        res = []
        for c in range(self.n_cores):
            d = {}
            for i, name in enumerate(self.out_names):
                arr = np.asarray(out[i]).reshape(
                    self.n_cores, *self.out_avals[i].shape)
                d[name] = arr[c]
            res.append(d)
        return res

    def time_runs(self, iters=10, warmup=2):
        import time
        for _ in range(warmup):
            self.run()
        times = []
        for _ in range(iters):
            t0 = time.perf_counter()
            self.run()
            times.append(time.perf_counter() - t0)
        return min(times), sum(times) / len(times)


_cached = {}


def _get_runner(time_reps: int = 1) -> BassRunner:
    if time_reps not in _cached:
        nc = build_kernel(time_reps)
        _cached[time_reps] = BassRunner(nc, N_CORES)
    return _cached[time_reps]


def _in_maps(inputs: dict) -> list:
    shared = {k: np.ascontiguousarray(np.asarray(v, dtype=np.float32))
              for k, v in inputs.items() if k != "x"}
    x_full = np.ascontiguousarray(np.asarray(inputs["x"], dtype=np.float32))
    maps = []
    for c in range(N_CORES):
        m = dict(shared)
        m["x"] = x_full[c * TOK_PER_CORE:(c + 1) * TOK_PER_CORE]
        maps.append(m)
    return maps


def kernel(**inputs) -> np.ndarray:
    runner = _get_runner(1)
    runner.put_inputs(_in_maps(inputs))
    res = runner.results(runner.run())
    return np.concatenate([r["out"] for r in res], axis=0)


# revision 18
# speedup vs baseline: 1.4082x; 1.4082x over previous
"""MoE decoder Trainium2 kernel (nn_MoEDecoder_67654324846797).

Strategy
--------
Data-parallel: the token dim (N=65536) is sharded across 8 NeuronCores
(8192 tokens each); all weights are replicated. No collectives.

Per-core kernel (feature-major, weight-stationary, f32r matmuls):
  - x tiles are loaded token-major and transposed on the PE (16x [128,128]
    per 512-token tile) into feature-major xT [512, 512tok].
  - All matmuls run with weights as the stationary operand in float32r
    (fp32 storage rounded to 11 mantissa bits, fp32 PSUM accumulation;
    ~3e-4 end-to-end rel err, same speed as bf16 on this stack).
  - Gating: 3-layer MLP -> logits l.T [8, 512] in PSUM; softmax is done
    without max subtraction (logits for this model are in [-0.5, 0.35]):
    exp on ACT, sum over experts via a ones-matmul, reciprocal on DVE,
    probs p = exp * (1/Z) on DVE.
  - Engines can't read 0-stride partition APs, so the per-token gate probs
    are broadcast across partitions by bouncing p through a DRAM scratch
    and DMA-loading with a 0-stride (legal on the DRAM side): one [128,512]
    f32r tile per expert.
  - The broadcast chain has ~10us of DMA latency, so the whole gating phase
    (A) runs 2 tiles ahead of the expert phase (B) in a software pipeline.
  - Experts: per 512-token tile, L1/L2 accumulate in PSUM with ACT doing
    bias+relu; h2 is scaled by the broadcast gate prob on DVE; the 8
    experts' L3 matmuls all accumulate into one PSUM bank pair, plus a
    K=8 matmul eb3.T @ probT for the gated bias.
  - The summed output is transposed back to token-major on the PE and
    DMA'd out.
  - DMA queues are split: x/out + most weights on the SP HWDGE ring,
    1/3 of expert weights on the ACT HWDGE ring and 1/3 on SWDGE, all
    broadcast/bounce DMAs on SWDGE, so no queue blocks another's critical
    path.

Measured (8 cores, this stack): ~590-620 us HW exec single-burst
(~700 us under sustained repeat due to P0 power throttling),
relative error ~3.3e-4 vs the fp32 reference.
"""

import numpy as np

import concourse.bass as bass
import concourse.tile as tile
from concourse import bacc, mybir
from concourse.masks import make_identity

F32 = mybir.dt.float32
F32R = mybir.dt.float32r
BF16 = mybir.dt.bfloat16

N_TOKENS = 65536
N_CORES = 8
TOK_PER_CORE = N_TOKENS // N_CORES  # 8192
TILE = 512  # tokens per tile
N_TILES = TOK_PER_CORE // TILE  # 16
IN_CH = 512
HID = 256
OUT_CH = 256
E = 8

RELU = mybir.ActivationFunctionType.Relu
EXP = mybir.ActivationFunctionType.Exp


def build_kernel(time_reps: int = 1) -> bass.Bass:
    """Build the per-core SPMD program. time_reps>1 wraps the main loop in a
    hardware repeat loop (same work each iteration) for timing."""
    nc = bacc.Bacc("TRN2", target_bir_lowering=False, debug=False,
                   num_devices=N_CORES)

    x = nc.dram_tensor("x", [TOK_PER_CORE, IN_CH], F32R, kind="ExternalInput").ap()
    eW1 = nc.dram_tensor("eW1", [E, IN_CH, HID], F32R, kind="ExternalInput").ap()
    eb1 = nc.dram_tensor("eb1", [E, HID], F32, kind="ExternalInput").ap()
    eW2 = nc.dram_tensor("eW2", [E, HID, HID], F32R, kind="ExternalInput").ap()
    eb2 = nc.dram_tensor("eb2", [E, HID], F32, kind="ExternalInput").ap()
    eW3 = nc.dram_tensor("eW3", [E, HID, OUT_CH], F32R, kind="ExternalInput").ap()
    eb3 = nc.dram_tensor("eb3", [E, OUT_CH], F32R, kind="ExternalInput").ap()
    gW1 = nc.dram_tensor("gW1", [IN_CH, HID], F32R, kind="ExternalInput").ap()
    gb1 = nc.dram_tensor("gb1", [HID], F32, kind="ExternalInput").ap()
    gW2 = nc.dram_tensor("gW2", [HID, HID], F32R, kind="ExternalInput").ap()
    gb2 = nc.dram_tensor("gb2", [HID], F32, kind="ExternalInput").ap()
    gW3 = nc.dram_tensor("gW3", [HID, E], F32R, kind="ExternalInput").ap()
    gb3 = nc.dram_tensor("gb3", [E], F32, kind="ExternalInput").ap()
    out = nc.dram_tensor("out", [TOK_PER_CORE, OUT_CH], F32, kind="ExternalOutput").ap()

    with tile.TileContext(nc) as tc:
        _body(nc, tc, x, eW1, eb1, eW2, eb2, eW3, eb3,
              gW1, gb1, gW2, gb2, gW3, gb3, out, time_reps)
    nc.compile()
    return nc


def _body(nc, tc, x, eW1, eb1, eW2, eb2, eW3, eb3,
          gW1, gb1, gW2, gb2, gW3, gb3, out, time_reps):
    from contextlib import ExitStack

    ctx = ExitStack()
    with ctx:
        wpool = ctx.enter_context(tc.tile_pool(name="wpool", bufs=1))
        io_pool = ctx.enter_context(tc.tile_pool(name="io", bufs=2))
        act_pool = ctx.enter_context(tc.tile_pool(name="act", bufs=2))
        small_pool = ctx.enter_context(tc.tile_pool(name="small", bufs=2))
        ps_mlp = ctx.enter_context(tc.tile_pool(name="ps_mlp", bufs=4, space="PSUM"))
        ps_out = ctx.enter_context(tc.tile_pool(name="ps_out", bufs=1, space="PSUM"))
        ps_tr = ctx.enter_context(tc.tile_pool(name="ps_tr", bufs=2, space="PSUM"))
        dram_pool = ctx.enter_context(tc.tile_pool(name="dram", bufs=3, space="DRAM"))

        # ---- prefetch x for tiles 0/1 so the weight stream doesn't delay
        # the first transposes/gating ----
        x_r0 = x.rearrange("(t s p) f -> t p s f", p=128, s=4)
        x_nat_t = {}

        def load_x(t):
            x_nat = io_pool.tile([128, 4, IN_CH], F32R, name="x_nat")
            nc.sync.dma_start(x_nat, x_r0[t])
            x_nat_t[t] = x_nat

        if time_reps == 1:
            load_x(0)
            load_x(1)

        # ---- weight preload (feature-major, stationary layouts) ----
        # Gating weights/biases first (needed earliest), then expert weights
        # interleaved per expert and spread over 3 DMA rings so tile 0's
        # compute starts while later experts' weights still stream.
        g1w = wpool.tile([128, 4, HID], F32R, name="g1w")
        nc.sync.dma_start(g1w, gW1.rearrange("(kt kp) m -> kp kt m", kp=128))
        g2w = wpool.tile([128, 2, HID], F32R, name="g2w")
        nc.sync.dma_start(g2w, gW2.rearrange("(kt kp) m -> kp kt m", kp=128))
        g3w = wpool.tile([128, 2, E], F32R, name="g3w")
        nc.sync.dma_start(g3w, gW3.rearrange("(kt kp) m -> kp kt m", kp=128))
        g1b = wpool.tile([128, 2], F32, name="g1b")
        nc.sync.dma_start(g1b, gb1.rearrange("(mt mp) -> mp mt", mp=128))
        g2b = wpool.tile([128, 2], F32, name="g2b")
        nc.sync.dma_start(g2b, gb2.rearrange("(mt mp) -> mp mt", mp=128))
        g3b = wpool.tile([E, 1], F32, name="g3b")
        nc.sync.dma_start(g3b, gb3.rearrange("(e one) -> e one", one=1))
        b1e = wpool.tile([128, E, 2], F32, name="b1e")
        nc.sync.dma_start(b1e, eb1.rearrange("e (mt mp) -> mp e mt", mp=128))
        b2e = wpool.tile([128, E, 2], F32, name="b2e")
        nc.sync.dma_start(b2e, eb2.rearrange("e (mt mp) -> mp e mt", mp=128))
        b3e = wpool.tile([E, OUT_CH], F32R, name="b3e")  # lhsT for bias matmul
        nc.sync.dma_start(b3e, eb3)
        w1e = wpool.tile([128, E, 4, HID], F32R, name="w1e")
        w2e = wpool.tile([128, E, 2, HID], F32R, name="w2e")
        w3e = wpool.tile([128, E, 2, OUT_CH], F32R, name="w3e")
        eW1r = eW1.rearrange("e (kt kp) m -> e kp kt m", kp=128)
        eW2r = eW2.rearrange("e (kt kp) m -> e kp kt m", kp=128)
        eW3r = eW3.rearrange("e (kt kp) m -> e kp kt m", kp=128)
        rings = [nc.sync, nc.scalar, nc.gpsimd]
        for e in range(E):
            ring = rings[e % 3]
            ring.dma_start(w1e[:, e], eW1r[e])
            ring.dma_start(w2e[:, e], eW2r[e])
            ring.dma_start(w3e[:, e], eW3r[e])

        identf = wpool.tile([128, 128], F32, name="identf")
        make_identity(nc, identf)
        identr = wpool.tile([128, 128], F32R, name="identr")
        nc.vector.tensor_copy(identr, identf)
        ones8 = wpool.tile([E, 1], F32, name="ones8")
        nc.vector.memset(ones8, 1.0)
        ones8r = wpool.tile([E, 1], F32R, name="ones8r")
        nc.vector.tensor_copy(ones8r, ones8)

        x_r = x.rearrange("(t s p) f -> t p s f", p=128, s=4)  # [16,128,4,512]
        out_r = out.rearrange("(t s p) o -> t p s o", p=128, s=4)

        # Pipelined 2-phase structure: phase A (load/transpose x, gating MLP,
        # probability broadcast DMA chain) runs 2 tiles ahead of phase B
        # (experts) so the w_bc DRAM-bounce latency is hidden behind B's PE
        # work.
        xT_t, wbc_t, probT_t = {}, {}, {}

        def phase_a(t):
            if t not in x_nat_t:
                load_x(t)
            x_nat = x_nat_t.pop(t)
            xT = act_pool.tile([128, 4, TILE], F32R, name="xT", bufs=3)
            for kt in range(4):
                p_tr = ps_tr.tile([128, TILE], F32R, name="p_tr", tag="ptr")
                for sj in range(4):
                    nc.tensor.transpose(
                        p_tr[:, sj * 128:(sj + 1) * 128],
                        x_nat[:, sj, kt * 128:(kt + 1) * 128], identr)
                nc.vector.tensor_copy(xT[:, kt, :], p_tr)

            g1T = act_pool.tile([128, 2, TILE], F32R, name="g1T", bufs=1)
            for mt in range(2):
                p_g = ps_mlp.tile([128, TILE], F32, name="p_g", tag="pmlp")
                for kt in range(4):
                    nc.tensor.matmul(p_g, g1w[:, kt, mt * 128:(mt + 1) * 128],
                                     xT[:, kt, :], start=(kt == 0), stop=(kt == 3))
                nc.scalar.activation(g1T[:, mt, :], p_g, RELU, bias=g1b[:, mt:mt + 1])
            g2T = act_pool.tile([128, 2, TILE], F32R, name="g2T", bufs=1)
            for mt in range(2):
                p_g2 = ps_mlp.tile([128, TILE], F32, name="p_g2", tag="pmlp")
                for kt in range(2):
                    nc.tensor.matmul(p_g2, g2w[:, kt, mt * 128:(mt + 1) * 128],
                                     g1T[:, kt, :], start=(kt == 0), stop=(kt == 1))
                nc.scalar.activation(g2T[:, mt, :], p_g2, RELU, bias=g2b[:, mt:mt + 1])
            p_l = ps_tr.tile([E, TILE], F32, name="p_l", tag="ptr")
            for kt in range(2):
                nc.tensor.matmul(p_l, g3w[:, kt, :], g2T[:, kt, :],
                                 start=(kt == 0), stop=(kt == 1))
            expT = small_pool.tile([E, TILE], F32R, name="expT")
            nc.scalar.activation(expT, p_l, EXP, bias=g3b)

            # Z = sum_e exp_e; r = 1/Z; prob = exp * r (normalized gate probs)
            p_z = ps_tr.tile([1, TILE], F32, name="p_z", tag="ptr")
            nc.tensor.matmul(p_z, ones8r, expT, start=True, stop=True)
            r_sb = small_pool.tile([1, TILE], F32, name="r_sb")
            nc.vector.reciprocal(r_sb, p_z)
            r_dram = dram_pool.tile([1, TILE], F32, name="r_dram")
            nc.gpsimd.dma_start(r_dram, r_sb)
            rb8 = small_pool.tile([E, TILE], F32, name="rb8")
            nc.gpsimd.dma_start(rb8, r_dram[0, :].partition_broadcast(E))
            probT = small_pool.tile([E, TILE], F32R, name="probT", bufs=3)
            nc.vector.tensor_mul(probT, expT, rb8)
            prob_dram = dram_pool.tile([E, TILE], F32R, name="prob_dram")
            nc.gpsimd.dma_start(prob_dram, probT)
            w_bc = []
            for e in range(E):
                wbe = act_pool.tile([128, TILE], F32R, name=f"wbe{e}", tag="wbc",
                                    bufs=6)
                nc.gpsimd.dma_start(
                    wbe, prob_dram[e, :].partition_broadcast(128))
                w_bc.append(wbe)
            xT_t[t], wbc_t[t], probT_t[t] = xT, w_bc, probT

        def phase_b(t):
            xT, w_bc, probT = xT_t.pop(t), wbc_t.pop(t), probT_t.pop(t)
            p_o = [ps_out.tile([128, TILE], F32, name=f"p_o{mt}", tag=f"po{mt}")
                   for mt in range(2)]
            for e in range(E):
                h1T = act_pool.tile([128, 2, TILE], F32R, name="h1T", bufs=3)
                for mt in range(2):
                    p_h = ps_mlp.tile([128, TILE], F32, name="p_h", tag="pmlp")
                    for kt in range(4):
                        nc.tensor.matmul(p_h, w1e[:, e, kt, mt * 128:(mt + 1) * 128],
                                         xT[:, kt, :], start=(kt == 0), stop=(kt == 3))
                    nc.scalar.activation(h1T[:, mt, :], p_h, RELU,
                                         bias=b1e[:, e, mt:mt + 1])
                h2s = act_pool.tile([128, 2, TILE], F32R, name="h2s")
                for mt in range(2):
                    p_h2 = ps_mlp.tile([128, TILE], F32, name="p_h2", tag="pmlp")
                    for kt in range(2):
                        nc.tensor.matmul(p_h2, w2e[:, e, kt, mt * 128:(mt + 1) * 128],
                                         h1T[:, kt, :], start=(kt == 0), stop=(kt == 1))
                    h2T = act_pool.tile([128, TILE], F32R, name="h2T", bufs=3)
                    nc.scalar.activation(h2T, p_h2, RELU, bias=b2e[:, e, mt:mt + 1])
                    nc.vector.tensor_mul(h2s[:, mt, :], h2T, w_bc[e])
                for mt in range(2):
                    for kt in range(2):
                        nc.tensor.matmul(p_o[mt], w3e[:, e, kt, mt * 128:(mt + 1) * 128],
                                         h2s[:, kt, :],
                                         start=(e == 0 and kt == 0), stop=False,
                                         skip_group_check=True)

            # gated bias: p_o[mt] += eb3.T[mt-slice] @ probT
            for mt in range(2):
                nc.tensor.matmul(p_o[mt], b3e[:, mt * 128:(mt + 1) * 128], probT,
                                 start=False, stop=True, skip_group_check=True)

            outT = act_pool.tile([128, 2, TILE], F32R, name="outT")
            for mt in range(2):
                nc.vector.tensor_copy(outT[:, mt, :], p_o[mt])
            out_tok = io_pool.tile([128, 4, OUT_CH], F32, name="out_tok")
            for s_ in range(4):
                p_ot = ps_out.tile([128, OUT_CH], F32, name="p_ot", tag=f"po{s_ % 2}")
                for mt in range(2):
                    nc.tensor.transpose(
                        p_ot[:, mt * 128:(mt + 1) * 128].bitcast(F32R),
                        outT[:, mt, s_ * 128:(s_ + 1) * 128], identr)
                nc.vector.tensor_copy(out_tok[:, s_, :], p_ot)
            nc.sync.dma_start(out_r[t], out_tok)

        def main_loop():
            if time_reps > 1:
                load_x(0)
                load_x(1)
            phase_a(0)
            phase_a(1)
            for t in range(N_TILES):
                if t + 2 < N_TILES:
                    phase_a(t + 2)
                phase_b(t)

        if time_reps > 1:
            with tc.For_i(0, time_reps, 1):
                main_loop()
        else:
            main_loop()


# ---------------------------------------------------------------------------
# PJRT runner (self-contained; mirrors concourse.bass2jax.run_bass_via_pjrt
# but keeps the jitted callable + device inputs for repeat timing)
# ---------------------------------------------------------------------------
class BassRunner:
    def __init__(self, nc: bass.Bass, n_cores: int = 8):
        import jax
        from jax.sharding import Mesh, PartitionSpec
        from jax.experimental.shard_map import shard_map
        from concourse.bass2jax import (
            _bass_exec_p, install_neuronx_cc_hook, partition_id_tensor,
        )

        install_neuronx_cc_hook()
        self.jax = jax
        self.nc = nc
        self.n_cores = n_cores
        partition_name = (
            nc.partition_id_tensor.name if nc.partition_id_tensor else None
        )

        in_names, out_names, out_avals, zero_shapes = [], [], [], []
        for alloc in nc.m.functions[0].allocations:
            if not isinstance(alloc, mybir.MemoryLocationSet):
                continue
            name = alloc.memorylocations[0].name
            if alloc.kind == "ExternalInput":
                if name != partition_name:
                    in_names.append(name)
            elif alloc.kind == "ExternalOutput":
                shape = tuple(alloc.tensor_shape)
                np_dt = mybir.dt.np(alloc.dtype)
                out_names.append(name)
                out_avals.append(jax.core.ShapedArray(shape, np_dt))
                zero_shapes.append((shape, np_dt))

        self.in_names, self.out_names = in_names, out_names
        self.out_avals, self.zero_shapes = out_avals, zero_shapes
        n_params, n_outs = len(in_names), len(out_names)
        bind_in_names = in_names + out_names
        if partition_name is not None:
            bind_in_names.append(partition_name)

        def _b(*args):
            operands = list(args)
            if partition_name is not None:
                operands.append(partition_id_tensor())
            return tuple(_bass_exec_p.bind(
                *operands,
                out_avals=tuple(out_avals),
                in_names=tuple(bind_in_names),
                out_names=tuple(out_names),
                lowering_input_output_aliases=(),
                sim_require_finite=True,
                sim_require_nnan=True,
                nc=nc,
            ))

        devices = jax.devices()[:n_cores]
        assert len(devices) == n_cores
        self.mesh = Mesh(np.asarray(devices), ("core",))
        self.pspec = PartitionSpec("core")
        in_specs = (self.pspec,) * (n_params + n_outs)
        out_specs = (self.pspec,) * n_outs
        self.sharded = jax.jit(
            shard_map(_b, mesh=self.mesh, in_specs=in_specs,
                      out_specs=out_specs, check_rep=False),
            keep_unused=True,
        )
        self._dev_in = None

    def put_inputs(self, in_maps):
        import jax
        concat = [
            np.concatenate([in_maps[c][n] for c in range(self.n_cores)], axis=0)
            for n in self.in_names
        ]
        zeros = [
            np.zeros((self.n_cores * s[0], *s[1:]), d) for s, d in self.zero_shapes
        ]
        sh = jax.sharding.NamedSharding(self.mesh, self.pspec)
        self._dev_in = [jax.device_put(a, sh) for a in concat + zeros]
        jax.block_until_ready(self._dev_in)

    def run(self):
        out = self.sharded(*self._dev_in)
        self.jax.block_until_ready(out)
        return out

    def results(self, out):
        res = []
        for c in range(self.n_cores):
            d = {}
            for i, name in enumerate(self.out_names):
                arr = np.asarray(out[i]).reshape(
                    self.n_cores, *self.out_avals[i].shape)
                d[name] = arr[c]
            res.append(d)
        return res

    def time_runs(self, iters=10, warmup=2):
        import time
        for _ in range(warmup):
            self.run()
        times = []
        for _ in range(iters):
            t0 = time.perf_counter()
            self.run()
            times.append(time.perf_counter() - t0)
        return min(times), sum(times) / len(times)


_cached = {}


def _get_runner(time_reps: int = 1) -> BassRunner:
    if time_reps not in _cached:
        nc = build_kernel(time_reps)
        _cached[time_reps] = BassRunner(nc, N_CORES)
    return _cached[time_reps]


def _in_maps(inputs: dict) -> list:
    shared = {k: np.ascontiguousarray(np.asarray(v, dtype=np.float32))
              for k, v in inputs.items() if k != "x"}
    x_full = np.ascontiguousarray(np.asarray(inputs["x"], dtype=np.float32))
    maps = []
    for c in range(N_CORES):
        m = dict(shared)
        m["x"] = x_full[c * TOK_PER_CORE:(c + 1) * TOK_PER_CORE]
        maps.append(m)
    return maps


def kernel(**inputs) -> np.ndarray:
    runner = _get_runner(1)
    runner.put_inputs(_in_maps(inputs))
    res = runner.results(runner.run())
    return np.concatenate([r["out"] for r in res], axis=0)



# revision 19
# speedup vs baseline: 1.4325x; 1.0173x over previous
"""MoE decoder Trainium2 kernel (nn_MoEDecoder_67654324846797).

Strategy
--------
Data-parallel: the token dim (N=65536) is sharded across 8 NeuronCores
(8192 tokens each); all weights are replicated. No collectives.

Per-core kernel (feature-major, weight-stationary, f32r matmuls):
  - x tiles are loaded token-major and transposed on the PE (16x [128,128]
    per 512-token tile) into feature-major xT [512, 512tok].
  - All matmuls run with weights as the stationary operand in float32r
    (fp32 storage rounded to 11 mantissa bits, fp32 PSUM accumulation;
    ~3e-4 end-to-end rel err, same speed as bf16 on this stack).
  - Gating: 3-layer MLP -> logits l.T [8, 512] in PSUM; softmax is done
    without max subtraction (logits for this model are in [-0.5, 0.35]):
    exp on ACT, sum over experts via a ones-matmul, reciprocal on DVE,
    probs p = exp * (1/Z) on DVE.
  - Engines can't read 0-stride partition APs, so the per-token gate probs
    are broadcast across partitions by bouncing p through a DRAM scratch
    and DMA-loading with a 0-stride (legal on the DRAM side): one [128,512]
    f32r tile per expert.
  - The broadcast chain has ~10us of DMA latency, so the whole gating phase
    (A) runs 2 tiles ahead of the expert phase (B) in a software pipeline.
  - Experts: per 512-token tile, L1/L2 accumulate in PSUM with ACT doing
    bias+relu; h2 is scaled by the broadcast gate prob on DVE; the 8
    experts' L3 matmuls all accumulate into one PSUM bank pair, plus a
    K=8 matmul eb3.T @ probT for the gated bias.
  - The summed output is transposed back to token-major on the PE and
    DMA'd out.
  - DMA queues are split: x/out + most weights on the SP HWDGE ring,
    1/3 of expert weights on the ACT HWDGE ring and 1/3 on SWDGE, all
    broadcast/bounce DMAs on SWDGE, so no queue blocks another's critical
    path.

Measured (8 cores, this stack): ~590-620 us HW exec single-burst
(~700 us under sustained repeat due to P0 power throttling),
relative error ~3.3e-4 vs the fp32 reference.
"""

import numpy as np

import concourse.bass as bass
import concourse.tile as tile
from concourse import bacc, mybir
from concourse.masks import make_identity

F32 = mybir.dt.float32
F32R = mybir.dt.float32r
BF16 = mybir.dt.bfloat16

N_TOKENS = 65536
N_CORES = 8
TOK_PER_CORE = N_TOKENS // N_CORES  # 8192
TILE = 512  # tokens per tile
N_TILES = TOK_PER_CORE // TILE  # 16
IN_CH = 512
HID = 256
OUT_CH = 256
E = 8

RELU = mybir.ActivationFunctionType.Relu
EXP = mybir.ActivationFunctionType.Exp


def build_kernel(time_reps: int = 1) -> bass.Bass:
    """Build the per-core SPMD program. time_reps>1 wraps the main loop in a
    hardware repeat loop (same work each iteration) for timing."""
    nc = bacc.Bacc("TRN2", target_bir_lowering=False, debug=False,
                   num_devices=N_CORES)

    x = nc.dram_tensor("x", [TOK_PER_CORE, IN_CH], F32R, kind="ExternalInput").ap()
    eW1 = nc.dram_tensor("eW1", [E, IN_CH, HID], F32R, kind="ExternalInput").ap()
    eb1 = nc.dram_tensor("eb1", [E, HID], F32, kind="ExternalInput").ap()
    eW2 = nc.dram_tensor("eW2", [E, HID, HID], F32R, kind="ExternalInput").ap()
    eb2 = nc.dram_tensor("eb2", [E, HID], F32, kind="ExternalInput").ap()
    eW3 = nc.dram_tensor("eW3", [E, HID, OUT_CH], F32R, kind="ExternalInput").ap()
    eb3 = nc.dram_tensor("eb3", [E, OUT_CH], F32R, kind="ExternalInput").ap()
    gW1 = nc.dram_tensor("gW1", [IN_CH, HID], F32R, kind="ExternalInput").ap()
    gb1 = nc.dram_tensor("gb1", [HID], F32, kind="ExternalInput").ap()
    gW2 = nc.dram_tensor("gW2", [HID, HID], F32R, kind="ExternalInput").ap()
    gb2 = nc.dram_tensor("gb2", [HID], F32, kind="ExternalInput").ap()
    gW3 = nc.dram_tensor("gW3", [HID, E], F32R, kind="ExternalInput").ap()
    gb3 = nc.dram_tensor("gb3", [E], F32, kind="ExternalInput").ap()
    out = nc.dram_tensor("out", [TOK_PER_CORE, OUT_CH], F32, kind="ExternalOutput").ap()

    with tile.TileContext(nc) as tc:
        _body(nc, tc, x, eW1, eb1, eW2, eb2, eW3, eb3,
              gW1, gb1, gW2, gb2, gW3, gb3, out, time_reps)
    nc.compile()
    return nc


def _body(nc, tc, x, eW1, eb1, eW2, eb2, eW3, eb3,
          gW1, gb1, gW2, gb2, gW3, gb3, out, time_reps):
    from contextlib import ExitStack

    ctx = ExitStack()
    with ctx:
        wpool = ctx.enter_context(tc.tile_pool(name="wpool", bufs=1))
        io_pool = ctx.enter_context(tc.tile_pool(name="io", bufs=2))
        act_pool = ctx.enter_context(tc.tile_pool(name="act", bufs=2))
        small_pool = ctx.enter_context(tc.tile_pool(name="small", bufs=2))
        ps_mlp = ctx.enter_context(tc.tile_pool(name="ps_mlp", bufs=4, space="PSUM"))
        ps_out = ctx.enter_context(tc.tile_pool(name="ps_out", bufs=1, space="PSUM"))
        ps_tr = ctx.enter_context(tc.tile_pool(name="ps_tr", bufs=2, space="PSUM"))
        dram_pool = ctx.enter_context(tc.tile_pool(name="dram", bufs=3, space="DRAM"))

        # ---- prefetch x for tiles 0/1 so the weight stream doesn't delay
        # the first transposes/gating ----
        x_r0 = x.rearrange("(t s p) f -> t p s f", p=128, s=4)
        x_nat_t = {}

        def load_x(t):
            x_nat = io_pool.tile([128, 4, IN_CH], F32R, name="x_nat")
            nc.sync.dma_start(x_nat, x_r0[t])
            x_nat_t[t] = x_nat

        if time_reps == 1:
            load_x(0)
            load_x(1)

        # ---- weight preload (feature-major, stationary layouts) ----
        # Gating weights/biases first (needed earliest), then expert weights
        # interleaved per expert and spread over 3 DMA rings so tile 0's
        # compute starts while later experts' weights still stream.
        g1w = wpool.tile([128, 4, HID], F32R, name="g1w")
        nc.sync.dma_start(g1w, gW1.rearrange("(kt kp) m -> kp kt m", kp=128))
        g2w = wpool.tile([128, 2, HID], F32R, name="g2w")
        nc.sync.dma_start(g2w, gW2.rearrange("(kt kp) m -> kp kt m", kp=128))
        g3w = wpool.tile([128, 2, E], F32R, name="g3w")
        nc.sync.dma_start(g3w, gW3.rearrange("(kt kp) m -> kp kt m", kp=128))
        g1b = wpool.tile([128, 2], F32, name="g1b")
        nc.sync.dma_start(g1b, gb1.rearrange("(mt mp) -> mp mt", mp=128))
        g2b = wpool.tile([128, 2], F32, name="g2b")
        nc.sync.dma_start(g2b, gb2.rearrange("(mt mp) -> mp mt", mp=128))
        g3b = wpool.tile([E, 1], F32, name="g3b")
        nc.sync.dma_start(g3b, gb3.rearrange("(e one) -> e one", one=1))
        b1e = wpool.tile([128, E, 2], F32, name="b1e")
        nc.sync.dma_start(b1e, eb1.rearrange("e (mt mp) -> mp e mt", mp=128))
        b2e = wpool.tile([128, E, 2], F32, name="b2e")
        nc.sync.dma_start(b2e, eb2.rearrange("e (mt mp) -> mp e mt", mp=128))
        b3e = wpool.tile([E, OUT_CH], F32R, name="b3e")  # lhsT for bias matmul
        nc.sync.dma_start(b3e, eb3)
        w1e = wpool.tile([128, E, 4, HID], F32R, name="w1e")
        w2e = wpool.tile([128, E, 2, HID], F32R, name="w2e")
        w3e = wpool.tile([128, E, 2, OUT_CH], F32R, name="w3e")
        eW1r = eW1.rearrange("e (kt kp) m -> e kp kt m", kp=128)
        eW2r = eW2.rearrange("e (kt kp) m -> e kp kt m", kp=128)
        eW3r = eW3.rearrange("e (kt kp) m -> e kp kt m", kp=128)
        rings = [nc.sync, nc.scalar, nc.gpsimd]
        for e in range(E):
            ring = rings[e % 3]
            ring.dma_start(w1e[:, e], eW1r[e])
            ring.dma_start(w2e[:, e], eW2r[e])
            ring.dma_start(w3e[:, e], eW3r[e])

        identf = wpool.tile([128, 128], F32, name="identf")
        make_identity(nc, identf)
        identr = wpool.tile([128, 128], F32R, name="identr")
        nc.vector.tensor_copy(identr, identf)
        ones8 = wpool.tile([E, 1], F32, name="ones8")
        nc.vector.memset(ones8, 1.0)
        ones8r = wpool.tile([E, 1], F32R, name="ones8r")
        nc.vector.tensor_copy(ones8r, ones8)

        x_r = x.rearrange("(t s p) f -> t p s f", p=128, s=4)  # [16,128,4,512]
        out_r = out.rearrange("(t s p) o -> t p s o", p=128, s=4)

        # Pipelined 2-phase structure: phase A (load/transpose x, gating MLP,
        # probability broadcast DMA chain) runs 2 tiles ahead of phase B
        # (experts) so the w_bc DRAM-bounce latency is hidden behind B's PE
        # work.
        xT_t, wbc_t, probT_t = {}, {}, {}

        def phase_a(t):
            if t not in x_nat_t:
                load_x(t)
            x_nat = x_nat_t.pop(t)
            xT = act_pool.tile([128, 4, TILE], F32R, name="xT", bufs=3)
            for kt in range(4):
                p_tr = ps_tr.tile([128, TILE], F32R, name="p_tr", tag="ptr")
                for sj in range(4):
                    nc.tensor.transpose(
                        p_tr[:, sj * 128:(sj + 1) * 128],
                        x_nat[:, sj, kt * 128:(kt + 1) * 128], identr)
                nc.vector.tensor_copy(xT[:, kt, :], p_tr)

            g1T = act_pool.tile([128, 2, TILE], F32R, name="g1T", bufs=1)
            for mt in range(2):
                p_g = ps_mlp.tile([128, TILE], F32, name="p_g", tag="pmlp")
                for kt in range(4):
                    nc.tensor.matmul(p_g, g1w[:, kt, mt * 128:(mt + 1) * 128],
                                     xT[:, kt, :], start=(kt == 0), stop=(kt == 3))
                nc.scalar.activation(g1T[:, mt, :], p_g, RELU, bias=g1b[:, mt:mt + 1])
            g2T = act_pool.tile([128, 2, TILE], F32R, name="g2T", bufs=1)
            for mt in range(2):
                p_g2 = ps_mlp.tile([128, TILE], F32, name="p_g2", tag="pmlp")
                for kt in range(2):
                    nc.tensor.matmul(p_g2, g2w[:, kt, mt * 128:(mt + 1) * 128],
                                     g1T[:, kt, :], start=(kt == 0), stop=(kt == 1))
                nc.scalar.activation(g2T[:, mt, :], p_g2, RELU, bias=g2b[:, mt:mt + 1])
            p_l = ps_tr.tile([E, TILE], F32, name="p_l", tag="ptr")
            for kt in range(2):
                nc.tensor.matmul(p_l, g3w[:, kt, :], g2T[:, kt, :],
                                 start=(kt == 0), stop=(kt == 1))
            expT = small_pool.tile([E, TILE], F32R, name="expT")
            nc.scalar.activation(expT, p_l, EXP, bias=g3b)

            # Z = sum_e exp_e; r = 1/Z; prob = exp * r (normalized gate probs)
            p_z = ps_tr.tile([1, TILE], F32, name="p_z", tag="ptr")
            nc.tensor.matmul(p_z, ones8r, expT, start=True, stop=True)
            r_sb = small_pool.tile([1, TILE], F32, name="r_sb")
            nc.vector.reciprocal(r_sb, p_z)
            r_dram = dram_pool.tile([1, TILE], F32, name="r_dram")
            nc.gpsimd.dma_start(r_dram, r_sb)
            rb8 = small_pool.tile([E, TILE], F32, name="rb8")
            nc.gpsimd.dma_start(rb8, r_dram[0, :].partition_broadcast(E))
            probT = small_pool.tile([E, TILE], F32R, name="probT", bufs=3)
            nc.vector.tensor_mul(probT, expT, rb8)
            prob_dram = dram_pool.tile([E, TILE], F32R, name="prob_dram")
            nc.gpsimd.dma_start(prob_dram, probT)
            w_bc = []
            for e in range(E):
                wbe = act_pool.tile([128, TILE], F32R, name=f"wbe{e}", tag="wbc",
                                    bufs=6)
                nc.gpsimd.dma_start(
                    wbe, prob_dram[e, :].partition_broadcast(128))
                w_bc.append(wbe)
            xT_t[t], wbc_t[t], probT_t[t] = xT, w_bc, probT

        def phase_b(t):
            xT, w_bc, probT = xT_t.pop(t), wbc_t.pop(t), probT_t.pop(t)
            p_o = [ps_out.tile([128, TILE], F32, name=f"p_o{mt}", tag=f"po{mt}")
                   for mt in range(2)]
            for e in range(E):
                h1T = act_pool.tile([128, 2, TILE], F32R, name="h1T", bufs=3)
                for mt in range(2):
                    p_h = ps_mlp.tile([128, TILE], F32, name="p_h", tag="pmlp")
                    for kt in range(4):
                        nc.tensor.matmul(p_h, w1e[:, e, kt, mt * 128:(mt + 1) * 128],
                                         xT[:, kt, :], start=(kt == 0), stop=(kt == 3))
                    nc.vector.tensor_scalar(
                        h1T[:, mt, :], p_h, b1e[:, e, mt:mt + 1], 0.0,
                        mybir.AluOpType.add, mybir.AluOpType.max)
                h2s = act_pool.tile([128, 2, TILE], F32R, name="h2s")
                for mt in range(2):
                    p_h2 = ps_mlp.tile([128, TILE], F32, name="p_h2", tag="pmlp")
                    for kt in range(2):
                        nc.tensor.matmul(p_h2, w2e[:, e, kt, mt * 128:(mt + 1) * 128],
                                         h1T[:, kt, :], start=(kt == 0), stop=(kt == 1))
                    h2T = act_pool.tile([128, TILE], F32R, name="h2T", bufs=3)
                    nc.scalar.activation(h2T, p_h2, RELU, bias=b2e[:, e, mt:mt + 1])
                    nc.vector.tensor_mul(h2s[:, mt, :], h2T, w_bc[e])
                for mt in range(2):
                    for kt in range(2):
                        nc.tensor.matmul(p_o[mt], w3e[:, e, kt, mt * 128:(mt + 1) * 128],
                                         h2s[:, kt, :],
                                         start=(e == 0 and kt == 0), stop=False,
                                         skip_group_check=True)

            # gated bias: p_o[mt] += eb3.T[mt-slice] @ probT
            for mt in range(2):
                nc.tensor.matmul(p_o[mt], b3e[:, mt * 128:(mt + 1) * 128], probT,
                                 start=False, stop=True, skip_group_check=True)

            outT = act_pool.tile([128, 2, TILE], F32R, name="outT")
            for mt in range(2):
                nc.vector.tensor_copy(outT[:, mt, :], p_o[mt])
            out_tok = io_pool.tile([128, 4, OUT_CH], F32, name="out_tok")
            for s_ in range(4):
                p_ot = ps_out.tile([128, OUT_CH], F32, name="p_ot", tag=f"po{s_ % 2}")
                for mt in range(2):
                    nc.tensor.transpose(
                        p_ot[:, mt * 128:(mt + 1) * 128].bitcast(F32R),
                        outT[:, mt, s_ * 128:(s_ + 1) * 128], identr)
                nc.vector.tensor_copy(out_tok[:, s_, :], p_ot)
            nc.sync.dma_start(out_r[t], out_tok)

        def main_loop():
            if time_reps > 1:
                load_x(0)
                load_x(1)
            phase_a(0)
            phase_a(1)
            for t in range(N_TILES):
                if t + 2 < N_TILES:
                    phase_a(t + 2)
                phase_b(t)

        if time_reps > 1:
            with tc.For_i(0, time_reps, 1):
                main_loop()
        else:
            main_loop()


# ---------------------------------------------------------------------------
# PJRT runner (self-contained; mirrors concourse.bass2jax.run_bass_via_pjrt
# but keeps the jitted callable + device inputs for repeat timing)
# ---------------------------------------------------------------------------
class BassRunner:
    def __init__(self, nc: bass.Bass, n_cores: int = 8):
        import jax
        from jax.sharding import Mesh, PartitionSpec
        from jax.experimental.shard_map import shard_map
        from concourse.bass2jax import (
            _bass_exec_p, install_neuronx_cc_hook, partition_id_tensor,
        )

        install_neuronx_cc_hook()
        self.jax = jax
        self.nc = nc
        self.n_cores = n_cores
        partition_name = (
            nc.partition_id_tensor.name if nc.partition_id_tensor else None
        )

        in_names, out_names, out_avals, zero_shapes = [], [], [], []
        for alloc in nc.m.functions[0].allocations:
            if not isinstance(alloc, mybir.MemoryLocationSet):
                continue
            name = alloc.memorylocations[0].name
            if alloc.kind == "ExternalInput":
                if name != partition_name:
                    in_names.append(name)
            elif alloc.kind == "ExternalOutput":
                shape = tuple(alloc.tensor_shape)
                np_dt = mybir.dt.np(alloc.dtype)
                out_names.append(name)
                out_avals.append(jax.core.ShapedArray(shape, np_dt))
                zero_shapes.append((shape, np_dt))

        self.in_names, self.out_names = in_names, out_names
        self.out_avals, self.zero_shapes = out_avals, zero_shapes
        n_params, n_outs = len(in_names), len(out_names)
        bind_in_names = in_names + out_names
        if partition_name is not None:
            bind_in_names.append(partition_name)

        def _b(*args):
            operands = list(args)
            if partition_name is not None:
                operands.append(partition_id_tensor())
            return tuple(_bass_exec_p.bind(
                *operands,
                out_avals=tuple(out_avals),
                in_names=tuple(bind_in_names),
                out_names=tuple(out_names),
                lowering_input_output_aliases=(),
                sim_require_finite=True,
                sim_require_nnan=True,
                nc=nc,
            ))

        devices = jax.devices()[:n_cores]
        assert len(devices) == n_cores
        self.mesh = Mesh(np.asarray(devices), ("core",))
        self.pspec = PartitionSpec("core")
        in_specs = (self.pspec,) * (n_params + n_outs)
        out_specs = (self.pspec,) * n_outs
        self.sharded = jax.jit(
            shard_map(_b, mesh=self.mesh, in_specs=in_specs,
                      out_specs=out_specs, check_rep=False),
            keep_unused=True,
        )
        self._dev_in = None

    def put_inputs(self, in_maps):
        import jax
        concat = [
            np.concatenate([in_maps[c][n] for c in range(self.n_cores)], axis=0)
            for n in self.in_names
        ]
        zeros = [
            np.zeros((self.n_cores * s[0], *s[1:]), d) for s, d in self.zero_shapes
        ]
        sh = jax.sharding.NamedSharding(self.mesh, self.pspec)
        self._dev_in = [jax.device_put(a, sh) for a in concat + zeros]
        jax.block_until_ready(self._dev_in)

    def run(self):
        out = self.sharded(*self._dev_in)
        self.jax.block_until_ready(out)
        return out

    def results(self, out):
        res = []
        for c in range(self.n_cores):
            d = {}
            for i, name in enumerate(self.out_names):
                arr = np.asarray(out[i]).reshape(
                    self.n_cores, *self.out_avals[i].shape)
                d[name] = arr[c]
            res.append(d)
        return res

    def time_runs(self, iters=10, warmup=2):
        import time
        for _ in range(warmup):
            self.run()
        times = []
        for _ in range(iters):
            t0 = time.perf_counter()
            self.run()
            times.append(time.perf_counter() - t0)
        return min(times), sum(times) / len(times)


_cached = {}


def _get_runner(time_reps: int = 1) -> BassRunner:
    if time_reps not in _cached:
        nc = build_kernel(time_reps)
        _cached[time_reps] = BassRunner(nc, N_CORES)
    return _cached[time_reps]


def _in_maps(inputs: dict) -> list:
    shared = {k: np.ascontiguousarray(np.asarray(v, dtype=np.float32))
              for k, v in inputs.items() if k != "x"}
    x_full = np.ascontiguousarray(np.asarray(inputs["x"], dtype=np.float32))
    maps = []
    for c in range(N_CORES):
        m = dict(shared)
        m["x"] = x_full[c * TOK_PER_CORE:(c + 1) * TOK_PER_CORE]
        maps.append(m)
    return maps


def kernel(**inputs) -> np.ndarray:
    runner = _get_runner(1)
    runner.put_inputs(_in_maps(inputs))
    res = runner.results(runner.run())
    return np.concatenate([r["out"] for r in res], axis=0)

